# revision 20
# baseline (speedup 1.0000x reference)
"""Trainium2 Bass kernel for batched dense attention with elementwise affine
projections.

Problem (per batch element b, B=8, L=D=1024):
    wq = relu(q * Wq + bq)        # elementwise, Wq broadcast over batch
    wk = relu(k * Wk + bk)
    wv = relu(v * Wv + bv)
    S  = (wq @ wk.T) / sqrt(D)    # [L, L]
    A  = softmax(S, axis=-1)
    out = A @ wv                  # [L, D]

Sharding: data-parallel over batch. Core b computes batch element b
(B == n_cores == 8), no collectives.

Device strategy (per core):
  - q, k, Wq, Wk ship pre-transposed (D-major) in fp8e4m3; the score path
    tolerates fp8 (verified numerically: end-to-end error is dominated by
    the value path). This halves the head DMA, which paces kernel start.
    Inputs arrive as quarter-tensor DMAs (256 rows each via one 3D access
    pattern) to amortize the ~625ns/DMA HWDGE descriptor-generation cost
    while keeping fine-grained pipelining into the affines.
  - Affines run in D-major layout so the bias is a per-partition scalar:
    muls split across DVE and GpSimd (tunable), relu+bias on ScalarE/DVE
    (tunable), outputs packed into fp8 "pair" tiles [128, 2048] = two
    128-row K-chunks side by side - the [Ki, 2, dim] layout DoubleRow wants.
  - mm1 S^T[k, q] = (wkT).T @ (wqT): fp8 DoubleRow, K_eff=256/matmul.
  - exp(scale*S^T) -> bf16 on ScalarE straight out of PSUM (no
    max-subtraction: scores are ~0.009 +- 0.002 at this scale, exp is safe).
  - mm2 also runs fp8 DoubleRow via an exact decomposition that keeps the
    attention signal above fp8 resolution:
        out_un = (exp(S^T) - 1).T @ wv + colsum,   colsum[d] = sum_k wv[k,d]
        den[q] = sum_k (exp(S^T)[k,q] - 1) + L
    (exp-1) ~ 0.01 +- 0.002 lives in a fine fp8 binade, so the deviations
    survive quantization. colsum is a rank-1 correction added into the same
    PSUM accumulation as a K=1 outer-product matmul in float32r (full rate,
    full precision), and +L folds into the reciprocal.
  - The softmax denominator lands per-partition [q, 1] via N=1 ones-matmuls.
  - Final normalize is fused into PSUM->SBUF eviction as a per-partition
    reciprocal scale (ScalarE one 512-half, DVE the other).
  - Verified numerics vs fp32 reference: rel l2 err ~2.0e-3.
"""

import os
import numpy as np
import ml_dtypes
from contextlib import ExitStack

import concourse.bass as bass
import concourse.bacc as bacc
import concourse.mybir as mybir
import concourse.tile as tile
from concourse.bass_utils import run_bass_kernel_spmd

BF16 = mybir.dt.bfloat16
FP8 = mybir.dt.float8e4
F32 = mybir.dt.float32
F32R = mybir.dt.float32r
AF = mybir.ActivationFunctionType
ALU = mybir.AluOpType
DR = mybir.MatmulPerfMode.DoubleRow

B = 8          # batch == number of cores
L = 1024       # sequence length
D = 1024       # feature dim
P = 128        # partitions
NCH = L // P   # 8 chunks of 128 along the partitioned dim
NPR = NCH // 2  # 4 fp8 pair-chunks (256 contraction rows each)
NH = 2         # halves of 512 along the free dim (PSUM bank = 512 fp32)
H = 512
SCALE = 1.0 / 32.0  # 1/sqrt(D)

# Engine/layout balance knobs (tuned against the instruction cost model).
CFG = dict(
    dma_k_first=1,  # DMA order within each quarter: k-path before q-path
    k_pool=5,       # how many k-muls run on GpSimd (rest on DVE)
    vmul_pool=0,    # how many v-muls run on GpSimd
    vadd_pool=0,    # how many v-adds run on GpSimd
    vmax_pool=4,    # how many v-relu(max) ops run on GpSimd
    relu_dve=4,     # how many of the 16 q/k relus run on DVE (rest ScalarE)
    em1_act=4,      # how many of the 8 (exp-1) ops run on ScalarE (rest DVE)
    fin_dve=2,      # how many 512-halves of each final normalize run on DVE
    ps_bufs=3,      # PSUM tiles ([128,1024] = 2 banks each)
)

_CACHE: dict = {}
LAST_RESULT = None  # BassKernelResults of the most recent run (for test harness)


def build_nc(cfg=None):
    cfg = dict(CFG, **(cfg or {}))
    nc = bacc.Bacc(None, target_bir_lowering=False)

    qT = nc.declare_dram_parameter("qT", [D, L], FP8, isOutput=False)
    kT = nc.declare_dram_parameter("kT", [D, L], FP8, isOutput=False)
    wqT_d = nc.declare_dram_parameter("wqT", [D, L], FP8, isOutput=False)
    wkT_d = nc.declare_dram_parameter("wkT", [D, L], FP8, isOutput=False)
    v_d = nc.declare_dram_parameter("v", [L, D], BF16, isOutput=False)
    wv_d = nc.declare_dram_parameter("wv", [L, D], BF16, isOutput=False)
    bqk_d = nc.declare_dram_parameter("bqk", [P, 2 * NCH], F32, isOutput=False)
    bvb_d = nc.declare_dram_parameter("bvb", [P, D], BF16, isOutput=False)
    out_d = nc.declare_dram_parameter("out", [L, D], F32, isOutput=True)

    with tile.TileContext(nc) as tc, ExitStack() as ctx:
        qs_p = ctx.enter_context(tc.tile_pool(name="qs", bufs=4))
        ks_p = ctx.enter_context(tc.tile_pool(name="ks", bufs=4))
        qw_p = ctx.enter_context(tc.tile_pool(name="qws", bufs=4))
        kw_p = ctx.enter_context(tc.tile_pool(name="kws", bufs=4))
        vs_p = ctx.enter_context(tc.tile_pool(name="vs", bufs=2))
        vw_p = ctx.enter_context(tc.tile_pool(name="vws", bufs=2))
        tmp = ctx.enter_context(tc.tile_pool(name="tmp", bufs=6))
        exb_p = ctx.enter_context(tc.tile_pool(name="exb", bufs=3))
        pers = ctx.enter_context(tc.tile_pool(name="pers", bufs=1))
        wq_p = ctx.enter_context(tc.tile_pool(name="wq", bufs=NPR))
        wk_p = ctx.enter_context(tc.tile_pool(name="wk", bufs=NPR))
        wv_p = ctx.enter_context(tc.tile_pool(name="wv", bufs=NPR))
        em_p = ctx.enter_context(tc.tile_pool(name="em", bufs=NPR))
        outp = ctx.enter_context(tc.tile_pool(name="outp", bufs=3))
        recp = ctx.enter_context(tc.tile_pool(name="recp", bufs=3))
        psum = ctx.enter_context(
            tc.tile_pool(name="psum", bufs=cfg["ps_bufs"], space="PSUM")
        )
        pden = ctx.enter_context(tc.tile_pool(name="pden", bufs=1, space="PSUM"))

        # ---- input staging: quarter-tensor DMAs for the fp8 q/k path,
        # interleaved across tensors so pair C of every tensor lands together.
        def stage_quarter(pool, dram, tag, pr):
            t = pool.tile([P, 2 * L], FP8, tag=tag, name=f"{tag}{pr}")
            src = dram[pr * 2 * P : (pr + 1) * 2 * P, :].rearrange(
                "(c p) l -> p c l", p=P
            )
            nc.sync.dma_start(t[:].rearrange("p (c l) -> p c l", c=2), src)
            return t

        q_st, qw_st, k_st, kw_st = [], [], [], []
        for pr in range(NPR):
            if cfg.get("dma_k_first", 1):
                k_st.append(stage_quarter(ks_p, kT, "kst", pr))
                kw_st.append(stage_quarter(kw_p, wkT_d, "kwst", pr))
                q_st.append(stage_quarter(qs_p, qT, "qst", pr))
                qw_st.append(stage_quarter(qw_p, wqT_d, "qwst", pr))
            else:
                q_st.append(stage_quarter(qs_p, qT, "qst", pr))
                qw_st.append(stage_quarter(qw_p, wqT_d, "qwst", pr))
                k_st.append(stage_quarter(ks_p, kT, "kst", pr))
                kw_st.append(stage_quarter(kw_p, wkT_d, "kwst", pr))
            if pr == 0:
                bqk_sb = pers.tile([P, 2 * NCH], F32, tag="bqk")
                nc.sync.dma_start(bqk_sb[:], bqk_d[:])

        # v/Wv: bf16 half-tensor DMAs (value path keeps bf16 inputs)
        def stage_half(pool, dram, tag, h):
            t = pool.tile([P, 4 * L], BF16, tag=tag, name=f"{tag}{h}")
            src = dram[h * 4 * P : (h + 1) * 4 * P, :].rearrange(
                "(c p) l -> p c l", p=P
            )
            nc.sync.dma_start(t[:].rearrange("p (c l) -> p c l", c=4), src)
            return t

        bv_sb = pers.tile([P, D], BF16, tag="bv")
        nc.sync.dma_start(bv_sb[:], bvb_d[:])
        v_st, vw_st = [], []
        for h in range(2):
            v_st.append(stage_half(vs_p, v_d, "vst", h))
            vw_st.append(stage_half(vw_p, wv_d, "vwst", h))

        # constants for the matmul tricks
        ones8 = pers.tile([P, 2], FP8, tag="ones8")
        nc.vector.memset(ones8[:], 1.0)
        ones_f = pers.tile([1, P], F32, tag="ones_f")
        nc.vector.memset(ones_f[:], 1.0)
        ones_r = pers.tile([1, P], F32R, tag="ones_r")
        nc.scalar.activation(ones_r[:], ones_f[:], AF.Copy, bias=0.0, scale=1.0)
        neg1 = pers.tile([P, 1], F32, tag="neg1")
        nc.vector.memset(neg1[:], -1.0)
        cs_sb = pers.tile([1, D], F32R, tag="cs_sb")

        # fp8 pair tiles: pair C holds chunks 2C (cols 0:1024) and 2C+1
        # (cols 1024:2048) -> the [Ki, 2, dim] layout DoubleRow wants.
        def pairs(pool, tag):
            return [
                pool.tile([P, 2 * L], FP8, tag=tag, name=f"{tag}{i}")
                for i in range(NPR)
            ]

        wq_t, wk_t, wv_t, em_t = (
            pairs(wq_p, "wqt"),
            pairs(wk_p, "wkt"),
            pairs(wv_p, "wvt"),
            pairs(em_p, "emt"),
        )

        # ---- affine projections for q and k (transposed layout, per d-chunk)
        n_dve = cfg["relu_dve"]
        dve_set = {((2 * j + 1) * 16) // (2 * n_dve) for j in range(n_dve)}
        relu_n = 0
        for c in range(NCH):
            pr, sl = c // 2, slice((c % 2) * L, (c % 2 + 1) * L)

            def relu(dst, src, bias_col):
                nonlocal relu_n
                if relu_n in dve_set:
                    nc.vector.tensor_scalar(
                        dst, src, bqk_sb[:, bias_col : bias_col + 1], 0.0,
                        ALU.add, ALU.max,
                    )
                else:
                    nc.scalar.activation(
                        dst, src, AF.Relu,
                        bias=bqk_sb[:, bias_col : bias_col + 1], scale=1.0,
                    )
                relu_n += 1

            t0 = tmp.tile([P, L], BF16, tag="tmp", name=f"tq{c}")
            nc.vector.tensor_mul(t0[:], q_st[pr][:, sl], qw_st[pr][:, sl])
            relu(wq_t[pr][:, sl], t0[:], c)

            t1 = tmp.tile([P, L], BF16, tag="tmp", name=f"tk{c}")
            keng = nc.gpsimd if c < cfg["k_pool"] else nc.vector
            keng.tensor_mul(t1[:], k_st[pr][:, sl], kw_st[pr][:, sl])
            relu(wk_t[pr][:, sl], t1[:], NCH + c)

        # ---- affine projection for v (natural layout, bias along free dim)
        for c in range(NCH):
            hf, sl4 = c // 4, slice((c % 4) * L, (c % 4 + 1) * L)
            pr, sl = c // 2, slice((c % 2) * L, (c % 2 + 1) * L)
            t2 = tmp.tile([P, D], BF16, tag="vtmp", name=f"tv{c}")
            veng = nc.gpsimd if c < cfg["vmul_pool"] else nc.vector
            veng.tensor_mul(t2[:], v_st[hf][:, sl4], vw_st[hf][:, sl4])
            t3 = tmp.tile([P, D], BF16, tag="vtmp2", name=f"tv2{c}")
            aeng = nc.gpsimd if c < cfg["vadd_pool"] else nc.vector
            aeng.tensor_add(t3[:], t2[:], bv_sb[:])
            meng = nc.gpsimd if c < cfg.get("vmax_pool", 0) else nc.vector
            meng.tensor_scalar_max(wv_t[pr][:, sl], t3[:], 0.0)

        wq_3d = [t[:].rearrange("p (i l) -> p i l", i=2) for t in wq_t]
        wk_3d = [t[:].rearrange("p (i l) -> p i l", i=2) for t in wk_t]
        wv_3d = [t[:].rearrange("p (i l) -> p i l", i=2) for t in wv_t]
        em_3d = [t[:].rearrange("p (i l) -> p i l", i=2) for t in em_t]
        ones8_3d = ones8[:].rearrange("p (i o) -> p i o", i=2)

        # ---- mm1 (fp8 DoubleRow) + exp + (exp-1)->fp8 pairs
        for m in range(NCH):  # k-chunk (partition dim of S^T)
            ps = psum.tile([P, 2 * H], F32, tag="ps", name=f"ps{m}")
            for h in range(NH):  # q-half (free dim)
                for C in range(NPR):  # contraction over d (256 per matmul)
                    nc.tensor.matmul(
                        ps[:, h * H : (h + 1) * H],
                        wk_3d[C][:, :, m * P : (m + 1) * P],
                        wq_3d[C][:, :, h * H : (h + 1) * H],
                        start=(C == 0),
                        stop=(C == NPR - 1),
                        perf_mode=DR,
                    )
            ex_b = exb_p.tile([P, L], BF16, tag="exb", name=f"exb{m}")
            nc.scalar.activation(ex_b[:], ps[:], AF.Exp, bias=0.0, scale=SCALE)
            dst = em_t[m // 2][:, (m % 2) * L : (m % 2 + 1) * L]
            if m < cfg["em1_act"]:
                nc.scalar.activation(dst, ex_b[:], AF.Identity, bias=neg1[:], scale=1.0)
            else:
                nc.vector.tensor_scalar_add(dst, ex_b[:], -1.0)

        # ---- colsum[d] = sum_k wv[k, d] -> PSUM row -> SBUF f32r row
        # (plain fp8 matmuls: DR ldweights with a single-column weight fails
        # the walrus ISA check, so contract each pair half separately)
        pcs = psum.tile([1, 2 * H], F32, tag="ps", name="pcs")
        for h in range(NH):
            n = 0
            for C in range(NPR):
                for i in range(2):
                    nc.tensor.matmul(
                        pcs[:, h * H : (h + 1) * H],
                        ones8[:, 0:1],
                        wv_t[C][:, i * L + h * H : i * L + (h + 1) * H],
                        start=(n == 0),
                        stop=(n == 2 * NPR - 1),
                    )
                    n += 1
        nc.scalar.activation(cs_sb[:], pcs[:], AF.Copy, bias=0.0, scale=1.0)

        # ---- mm2 (fp8 DoubleRow) + rank-1 colsum correction + denominator
        #   out_un[q,d] = (exp-1)^T @ wv + ones x colsum
        #   den[q] = sum_k (exp-1) + L
        for qc in range(NCH):  # q-chunk (partition dim of out)
            po = psum.tile([P, 2 * H], F32, tag="ps", name=f"po{qc}")
            pd = pden.tile([P, 1], F32, tag="pd", name=f"pd{qc}")
            for C in range(NPR):  # contraction over k (256 per matmul)
                lhs = em_3d[C][:, :, qc * P : (qc + 1) * P]
                st = C == 0
                for h in range(NH):
                    nc.tensor.matmul(
                        po[:, h * H : (h + 1) * H],
                        lhs,
                        wv_3d[C][:, :, h * H : (h + 1) * H],
                        start=st,
                        stop=False,
                        perf_mode=DR,
                    )
                nc.tensor.matmul(
                    pd[:], lhs, ones8_3d[:, :, 0:1],
                    start=st, stop=(C == NPR - 1), perf_mode=DR,
                )
            for h in range(NH):  # rank-1 colsum correction, f32r full-rate
                nc.tensor.matmul(
                    po[:, h * H : (h + 1) * H],
                    ones_r[:],
                    cs_sb[:, h * H : (h + 1) * H],
                    start=False,
                    stop=True,
                )
            den = recp.tile([P, 1], F32, tag="den", name=f"den{qc}")
            nc.vector.tensor_scalar_add(den[:], pd[:], float(L))
            rec = recp.tile([P, 1], F32, tag="rec", name=f"rec{qc}")
            nc.vector.reciprocal(rec[:], den[:])
            ob = outp.tile([P, D], F32, tag="ob", name=f"ob{qc}")
            for h in range(NH):
                dst, src = ob[:, h * H : (h + 1) * H], po[:, h * H : (h + 1) * H]
                if cfg["fin_dve"] == 3:
                    use_dve = (2 * qc + h) % 2 == 0
                else:
                    use_dve = h < cfg["fin_dve"]
                if use_dve:
                    nc.vector.tensor_scalar_mul(dst, src, rec[:])
                else:
                    nc.scalar.activation(dst, src, AF.Copy, bias=0.0, scale=rec[:])
                nc.sync.dma_start(
                    out_d[qc * P : (qc + 1) * P, h * H : (h + 1) * H], dst
                )

    nc.finalize()
    return nc


def _get_nc():
    if "nc" not in _CACHE:
        _CACHE["nc"] = build_nc()
    return _CACHE["nc"]


def make_in_maps(query, key, value, Wq, bq, Wk, bk, Wv, bv):
    """Host-side sharding + layout prep: slice per batch, transpose the
    score-path operands to D-major, cast matmul operands to fp8/bf16."""
    bf = ml_dtypes.bfloat16
    f8 = ml_dtypes.float8_e4m3
    f32 = np.float32
    q = np.asarray(query, f32)
    k = np.asarray(key, f32)
    v = np.asarray(value, f32)
    wqT = np.ascontiguousarray(np.asarray(Wq, f32).T.astype(f8))
    wkT = np.ascontiguousarray(np.asarray(Wk, f32).T.astype(f8))
    wvn = np.ascontiguousarray(np.asarray(Wv, f32).astype(bf))
    bqk = np.concatenate(
        [
            np.asarray(bq, f32).reshape(NCH, P).T,
            np.asarray(bk, f32).reshape(NCH, P).T,
        ],
        axis=1,
    )
    bqk = np.ascontiguousarray(bqk)
    bvb = np.ascontiguousarray(
        np.broadcast_to(np.asarray(bv, f32).astype(bf)[None, :], (P, D))
    )
    in_maps = []
    for b in range(B):
        in_maps.append(
            {
                "qT": np.ascontiguousarray(q[b].T.astype(f8)),
                "kT": np.ascontiguousarray(k[b].T.astype(f8)),
                "wqT": wqT,
                "wkT": wkT,
                "v": np.ascontiguousarray(v[b].astype(bf)),
                "wv": wvn,
                "bqk": bqk,
                "bvb": bvb,
            }
        )
    return in_maps


def kernel(**inputs) -> np.ndarray:
    global LAST_RESULT
    nc = _get_nc()
    in_maps = make_in_maps(**inputs)
    res = run_bass_kernel_spmd(nc, in_maps, list(range(B)))
    LAST_RESULT = res
    return np.stack([res.results[i]["out"] for i in range(B)]).astype(np.float32)
